# revision 8
# baseline (speedup 1.0000x reference)
"""TRN2 Bass kernel for nn_KVGather: out[b,i,t] = kv[b, r_idx[b,i,t]] * r_weight[b,i,t].

Full shapes: r_idx/r_weight (32,49,4), kv (32,49,64,256) f32 -> out (32,49,4,64,256) f32.

Sharding: batch dim n=32 across 8 cores (4 batches/core), pure data parallel.

Per-core device kernel (memory-bound; HBM floor ~= (12.25 + 49) MB / 358 GB/s ~= 171 us):
  - Full KV shard (196 rows x 16384 f32) resident on SBUF partitions 0-63:
    partition p holds f32 elements [p*256, (p+1)*256) of every row, i.e. each
    row is 64 chunks of 1 KB. 1 KB DMA runs reach ~25 GB/s per SDMA engine
    (512 B runs are element-rate limited at ~19 GB/s).
  - Gather+scale: one [64, 256] f32 op per output tile. The source AP always
    has partition base 0 (dynamic-start APs silently drop a nonzero partition
    base on TRN2) with a host-provided row offset register; tiles of batches
    2-3 lane-shift their OUTPUT to staging partitions 64-127 so the store DMAs
    cover all 128 partitions and hence all 16 SDMA engines.
  - Host passes per-tile int32 free-dim offsets (= row*256) and a [64, 784]
    broadcast weight matrix; both are runtime data, the program is fixed.
"""

import os
import sys

sys.path.insert(0, "/opt/trn_rl_repo")

import numpy as np

N, P2, TOPK, HW_KV, C_KV = 32, 49, 4, 64, 256
NCORES = 8
NB = N // NCORES  # 4 batches per core
ROWS = NB * P2  # 196 kv rows per core
TILES = NB * P2 * TOPK  # 784 output tiles per core
HTILES = TILES // 2  # 392 tiles per output half
ROW_ELEMS = HW_KV * C_KV  # 16384 f32 per row/tile
HALF = 64  # partitions used by the kv table / per tile
CCH = ROW_ELEMS // HALF  # 256 f32 chunk per partition (1 KB)
PAIRS = 4  # tile pairs per staging buffer
NGROUPS = HTILES // PAIRS  # 98

# op order: group g, pair k, half h -> o = g*(2*PAIRS) + k*2 + h
# engine split DVE:ACT ~ 2:1
def _is_act(o):
    return o % 3 == 2


def _op_order():
    for g in range(NGROUPS):
        for k in range(PAIRS):
            i = g * PAIRS + k
            yield g * 2 * PAIRS + k * 2 + 0, 0, i
            yield g * 2 * PAIRS + k * 2 + 1, 1, i


_compiled = None


def _build():
    import concourse.bass as bass
    import concourse.tile as tile
    from concourse import bacc, mybir

    nc = bacc.Bacc(
        "TRN2",
        target_bir_lowering=False,
        debug=False,
        dynamic_dma_scratch_size=8192,
    )

    f32 = mybir.dt.float32
    i32 = mybir.dt.int32

    n_act = sum(1 for o, _, _ in _op_order() if _is_act(o))
    n_dve = TILES - n_act

    kv_d = nc.dram_tensor("kv", [ROWS, ROW_ELEMS], f32, kind="ExternalInput").ap()
    offs_dve_d = nc.dram_tensor("offs_dve", [1, n_dve], i32, kind="ExternalInput").ap()
    offs_act_d = nc.dram_tensor("offs_act", [1, n_act], i32, kind="ExternalInput").ap()
    wq_d = nc.dram_tensor("wq", [HALF, TILES], f32, kind="ExternalInput").ap()
    out_d = nc.dram_tensor("out", [TILES, ROW_ELEMS], f32, kind="ExternalOutput").ap()

    DVE = mybir.EngineType.DVE
    ACT = mybir.EngineType.Activation
    COPY = mybir.ActivationFunctionType.Copy
    MAX_OFF = (ROWS - 1) * CCH

    with tile.TileContext(nc) as tc:
        with (
            tc.tile_pool(name="resident", bufs=1) as res_pool,
            tc.tile_pool(name="stage", bufs=3) as stage_pool,
        ):
            kv_sb = res_pool.tile([HALF, ROWS * CCH], f32, tag="kv")
            offs_dve_sb = res_pool.tile([1, n_dve], i32, tag="offs_dve")
            offs_act_sb = res_pool.tile([1, n_act], i32, tag="offs_act")
            wq_sb = res_pool.tile([HALF, TILES], f32, tag="wq")

            nc.sync.dma_start(offs_dve_sb[:], offs_dve_d[:])
            nc.sync.dma_start(offs_act_sb[:], offs_act_d[:])
            nc.sync.dma_start(wq_sb[:], wq_d[:])

            # kv load: kv_sb[c64, r*256 + c] = kv[r, c64*256 + c]
            kv_dst = kv_sb[:].rearrange("p (r c) -> p r c", c=CCH)
            kv_src = kv_d.rearrange("r (p c) -> p r c", p=HALF)
            for q in range(4):
                rs = slice(q * (ROWS // 4), (q + 1) * (ROWS // 4))
                nc.sync.dma_start(kv_dst[:, rs, :], kv_src[:, rs, :])

            dpos = apos = 0
            op_list = list(_op_order())
            for g in range(NGROUPS):
                ops = op_list[g * 2 * PAIRS : (g + 1) * 2 * PAIRS]
                g_dve = [t for t in ops if not _is_act(t[0])]
                g_act = [t for t in ops if _is_act(t[0])]

                _, dve_vals = nc.values_load_multi_w_load_instructions(
                    offs_dve_sb[0:1, dpos : dpos + len(g_dve)],
                    engines=[DVE],
                    min_val=0,
                    max_val=MAX_OFF,
                    skip_runtime_bounds_check=True,
                )
                _, act_vals = nc.values_load_multi_w_load_instructions(
                    offs_act_sb[0:1, apos : apos + len(g_act)],
                    engines=[ACT],
                    min_val=0,
                    max_val=MAX_OFF,
                    skip_runtime_bounds_check=True,
                )
                dpos += len(g_dve)
                apos += len(g_act)
                vals = {}
                for (o, _, _), v in zip(g_dve, dve_vals):
                    vals[o] = v
                for (o, _, _), v in zip(g_act, act_vals):
                    vals[o] = v

                stage = stage_pool.tile([2 * HALF, PAIRS * CCH], f32, tag="st")
                for o, h, i in ops:
                    k = i - g * PAIRS
                    dst = stage[h * HALF : (h + 1) * HALF, k * CCH : (k + 1) * CCH]
                    src = kv_sb[0:HALF, bass.ds(vals[o], CCH)]
                    scale = wq_sb[0:HALF, (h * HTILES + i) : (h * HTILES + i) + 1]
                    if _is_act(o):
                        nc.scalar.activation(dst, src, COPY, scale=scale)
                    else:
                        nc.vector.tensor_scalar(
                            dst, src, scale, None, mybir.AluOpType.mult
                        )

                for h in range(2):
                    row0 = h * HTILES + g * PAIRS
                    dst = out_d[row0 : row0 + PAIRS, :].rearrange(
                        "jj (p c) -> p jj c", p=HALF
                    )
                    src = stage[h * HALF : (h + 1) * HALF, :].rearrange(
                        "p (jj c) -> p jj c", c=CCH
                    )
                    nc.sync.dma_start(dst, src)

    nc.compile()
    return nc


def _get_compiled():
    global _compiled
    if _compiled is None:
        _compiled = _build()
    return _compiled


def _enable_trace_hook():
    """Register the axon NTFF profile hook (missing antenv.axon_hooks shim)."""
    import types

    try:
        import antenv.axon_hooks  # noqa: F401

        return
    except ImportError:
        pass
    try:
        import antenv

        mod = types.ModuleType("antenv.axon_hooks")
        holder = {}
        mod.set_axon_ntff_profile_hook = lambda h: holder.__setitem__("h", h)
        mod.get_axon_ntff_profile_hook = lambda: holder.get("h")
        antenv.axon_hooks = mod
        sys.modules["antenv.axon_hooks"] = mod
        if "/root/.axon_site" not in sys.path:
            sys.path.insert(0, "/root/.axon_site")
        from trn_agent_boot.trn_boot import _ntff_profile_via_ctypes

        mod.set_axon_ntff_profile_hook(
            _ntff_profile_via_ctypes("/opt/axon/libaxon_pjrt.so")
        )

        import concourse.bass_utils as bu

        orig = bu.upload_artifacts

        def _safe_upload(tmpdir):
            try:
                return orig(tmpdir)
            except Exception:
                return tmpdir

        bu.upload_artifacts = _safe_upload
    except Exception as e:  # tracing is best-effort
        print(f"trace hook setup failed: {e}")


def kernel(r_idx, r_weight, kv):
    from concourse.bass_utils import run_bass_kernel_spmd

    r_idx = np.asarray(r_idx)
    r_weight = np.asarray(r_weight, dtype=np.float32)
    kv = np.ascontiguousarray(np.asarray(kv, dtype=np.float32))
    assert r_idx.shape == (N, P2, TOPK) and kv.shape == (N, P2, HW_KV, C_KV)

    nc = _get_compiled()

    order = list(_op_order())
    dve_ops = [(h, i) for (o, h, i) in order if not _is_act(o)]
    act_ops = [(h, i) for (o, h, i) in order if _is_act(o)]

    in_maps = []
    for c in range(NCORES):
        b0 = c * NB
        kv_shard = kv[b0 : b0 + NB].reshape(ROWS, ROW_ELEMS)
        idx_shard = np.asarray(r_idx[b0 : b0 + NB], dtype=np.int64)
        rows = (np.arange(NB)[:, None, None] * P2 + idx_shard).reshape(-1)
        offs = (rows * CCH).astype(np.int32)  # free-dim element offset per tile
        w_flat = r_weight[b0 : b0 + NB].reshape(-1).astype(np.float32)
        offs_dve = np.array([offs[h * HTILES + i] for h, i in dve_ops], np.int32)
        offs_act = np.array([offs[h * HTILES + i] for h, i in act_ops], np.int32)
        wq = np.ascontiguousarray(np.broadcast_to(w_flat, (HALF, TILES)))
        in_maps.append(
            {
                "kv": kv_shard,
                "offs_dve": np.ascontiguousarray(offs_dve[None, :]),
                "offs_act": np.ascontiguousarray(offs_act[None, :]),
                "wq": wq,
            }
        )

    trace = bool(int(os.environ.get("KV_TRACE", "0")))
    if trace:
        _enable_trace_hook()
    res = run_bass_kernel_spmd(nc, in_maps, list(range(NCORES)), trace=trace)

    if trace:
        kernel.last_exec_time_ns = res.exec_time_ns
        kernel.last_trace = (
            res.instructions_and_trace[1] if res.instructions_and_trace else None
        )

    out = np.empty((N, P2, TOPK, HW_KV, C_KV), dtype=np.float32)
    for c in range(NCORES):
        b0 = c * NB
        out[b0 : b0 + NB] = res.results[c]["out"].reshape(NB, P2, TOPK, HW_KV, C_KV)
    return out
